# revision 51
# baseline (speedup 1.0000x reference)
"""Grouped-scale dequant GEMM (AxCoreLinearFP16) on 8 Trainium2 NeuronCores.

y[b,s,o] = sum_i x[b,s,i] * (weight[o,i] * scales[o, i//128])

Strategy: data-parallel over the flattened (b*s) rows — each core gets a
[1024, 4096] x-shard and the full dequantized weight (no collectives).
The scales are folded into the weight on the host (a constant-folding
preprocessing step; np fp16 multiply rounds identically to the
reference's jnp fp16 multiply, so numerics are bit-equal), and all
tensors are pre-tiled on the host into the exact SBUF layout the PE
wants, so every device DMA is a large fully-contiguous transfer.

Device kernel (per core): a wall-to-wall PE matmul stream.
  - x^T resident in SBUF ([128, MT, KO, 128], 64 KiB/partition),
    loaded as 8 x 1 MiB contiguous chunks so m-tile 0 lands early.
  - w^T o-panels ([128, KO, 512], 32 KiB/partition) loaded two panels
    ahead (bufs=3) on their own DMA queue.
  - Per (o-panel, m-tile): 32 matmuls accumulate over the k-chunks
    into ONE PSUM bank ([128, 512] fp32), evicted by a casting ACT
    copy and DMA'd out. 4-bank rotation overlaps eviction.

HW-measured cost law on this toolchain (per N=512 matmul) that shaped
the design — CoreSim's cost model does not see any of these:
  - each matmul is a serial (InstLdweights, InstMatmult) pair; the
    stationary load is NOT hidden under the previous stream
    (~404 ns/MM baseline).
  - rewriting the ldweights AP from [[ps,128],[1,128]] to
    [[ps,128],[4,32],[1,4]] (src num_elems[2]==4) enables the ISA's
    %4 fast weight load, 4 cols/cycle: 404 -> ~341 ns/MM (-128 us).
  - consecutive matmuls that target different PSUM banks pay ~190 ns
    extra — so interleaved accumulation chains, stationary-sharing
    across two accumulators, and interleaved dequant broadcasts all
    LOSE on net. Keep each 32-matmul chain on one bank.
  - the winning form: drop the standalone InstLdweights and set
    ldweights=True on each InstMatmult (true self-loading matmul, ONE
    PE instruction per MM): ~341 -> ~325 ns/MM. Deleting ldweights
    withOUT setting the flag executes with stale weights (wrong
    results), and walrus --enable-ldw-opt=true miscompiles — both
    verified on HW.
  - on-device dequant (PE broadcast + DVE multiply, DEQUANT_ON_DEVICE)
    is kept as a working fallback but costs 150-250 us in bank
    switches + broadcast streams, hence the host fold.

Other toolchain workarounds:
  - walrus here accepts only ONE sync-wait per instruction: extra waits
    are peeled onto same-engine NoOps (_split_multiwait_insts)
  - InstPartitionBroadcast ("ISA wrong length") and broadcast-shaped
    DMAs (step-0 partition APs) are avoided.

Self-contained: hardcodes shapes from the problem spec.
"""

import sys

for _p in ("/opt/trn_rl_repo",):
    if _p not in sys.path:
        sys.path.insert(0, _p)

from contextlib import ExitStack

import numpy as np

import concourse.bass as bass
import concourse.mybir as mybir
import concourse.tile as tile
import bass_rust


FP16 = mybir.dt.float16
FP32 = mybir.dt.float32

P = 128
NCORES = 8
B, S, IN, OUT = 4, 2048, 4096, 4096
GROUP = 128
M = B * S // NCORES          # 1024 rows of x per core
KO = IN // P                 # 32 k-chunks == quant groups
OC = 512                     # o-chunk (matmul free dim)
NOC = OUT // OC              # 8
MT = M // P                  # 8 m-tiles

# True: dequant (scale broadcast + multiply) runs on-device, fused into the
# GEMM pipeline. False: scales are folded into the weight on the host
# (identical fp16 numerics to the reference's jnp fp16 multiply).
DEQUANT_ON_DEVICE = False

# Post-pass: drop an InstLdweights when it reloads the exact weights the PE
# already holds (consecutive matmuls sharing a stationary). HW-verified that
# matmuls use the last-loaded weights; walrus's own ldw-opt path and
# fully-fused self-loading matmuls both miscompute on this toolchain.
DEDUP_LDW = False     # stationary sharing forces PSUM bank alternation: SLOWER
LDW_OPT_FLAG = False  # walrus --enable-ldw-opt=true: WRONG results on HW
FUSE_LDW = True       # fuse + mm.ldweights=True: single self-loading matmult
LDW_FAST_AP = True    # rewrite ldweights APs to the %4 fast-weight-load form
THIN_MM_SEMS = False  # no measurable win; sem updates ride the commit path

_RUNNER = None


def _enable_ldw_opt_flag():
    """Patch concourse's walrus invocation to enable the LDW optimization
    (the flag is hardcoded off upstream; idempotent)."""
    from concourse import bass_utils
    if getattr(bass_utils.run_command, "_ldw_patched", False):
        return
    orig = bass_utils.run_command

    def run_command_ldw(cmd, **kw):
        cmd = ["--enable-ldw-opt=true" if c == "--enable-ldw-opt=false" else c
               for c in cmd]
        return orig(cmd, **kw)

    run_command_ldw._ldw_patched = True
    bass_utils.run_command = run_command_ldw


def _fuse_ldweights(nc):
    """Drop the standalone InstLdweights that bass splits off each matmul
    (their waits/updates fold into the next same-engine instruction — the
    matmult) and set ldweights=True on every matmult so it performs its own
    weight load (one PE instruction per matmul instead of two)."""
    ndrop = 0
    for f in nc.m.functions:
        for bb in f.blocks:
            new = []
            # pending syncs per engine: the scheduled instruction list
            # interleaves all engines, so a dropped ldweights' waits must
            # land on the next instruction of the SAME engine (its matmult).
            pend = {}
            for inst in bb.instructions:
                tn = type(inst).__name__
                if tn == "InstLdweights":
                    si = inst.sync_info
                    if si is not None and (si.on_wait or si.on_update):
                        w, u = pend.setdefault(inst.engine, ([], []))
                        w += list(si.on_wait or [])
                        u += list(si.on_update or [])
                    ndrop += 1
                    continue
                if tn == "InstMatmult":
                    inst.ldweights = True
                if inst.engine in pend:
                    pw, pu = pend.pop(inst.engine)
                    si = inst.sync_info
                    w = list(si.on_wait or []) if si else []
                    u = list(si.on_update or []) if si else []
                    inst.sync_info = bass_rust.SyncInfo(
                        on_wait=pw + w, on_update=pu + u)
                new.append(inst)
            assert not pend, "trailing ldweights sync"
            bb.instructions = new
    return ndrop


def _thin_mm_sem_updates(nc):
    """Coalesce the per-matmul progress-semaphore increments: drop the +1 on
    non-stop matmuls and make each chain's stop matmul increment by the
    accumulated count. Cumulative semaphore values at every kept update are
    exactly preserved, so all wait thresholds remain correct (a threshold
    that fell mid-chain is now satisfied at the chain's stop — later, never
    earlier). Removes ~97% of PE semaphore-update work."""
    from collections import Counter
    ids = Counter()
    for f in nc.m.functions:
        for bb in f.blocks:
            for inst in bb.instructions:
                if type(inst).__name__ == "InstMatmult":
                    si = inst.sync_info
                    if si is not None:
                        for u in (si.on_update or []):
                            if u.update_mode == "sem-inc" and u.update_value == 1:
                                ids[u.id] += 1
    if not ids:
        return 0
    sem_id = ids.most_common(1)[0][0]
    nthin = 0
    for f in nc.m.functions:
        for bb in f.blocks:
            acc = 0
            last_kept = None
            for inst in bb.instructions:
                if type(inst).__name__ != "InstMatmult":
                    continue
                si = inst.sync_info
                if si is None:
                    continue
                ups = list(si.on_update or [])
                tgt = [u for u in ups
                       if u.id == sem_id and u.update_mode == "sem-inc"
                       and u.update_value == 1]
                if not tgt:
                    continue
                rest = [u for u in ups
                        if not (u.id == sem_id and u.update_mode == "sem-inc"
                                and u.update_value == 1)]
                if inst.stop_tensor_calc:
                    t = tgt[0]
                    # 'sem-inc' bumps by 1 regardless of value; use
                    # 'sem-add-imm' (the mode DMA +=16 updates use) for +N
                    rest.append(bass_rust.SyncUpdate(
                        sync_type=t.sync_type, id=t.id, ant_name=t.ant_name,
                        update_mode="sem-add-imm", update_value=acc + 1,
                        update_reg=None))
                    acc = 0
                    last_kept = inst
                else:
                    acc += 1
                    nthin += 1
                inst.sync_info = bass_rust.SyncInfo(
                    on_wait=list(si.on_wait or []), on_update=rest)
            assert acc == 0, f"unflushed {acc} increments (no trailing stop)"
    return nthin


def _ap_sig(ap):
    return (ap.memref, ap.offset, str(ap.ap), str(ap.dtype))


def _fast_ldweights_aps(nc):
    """Rewrite each PE InstLdweights weights AP from [[pstride, 128], [1, C]]
    to [[pstride, 128], [4, C/4], [1, 4]]. Same element stream, but
    src_mem_pattern.num_elems[2] == 4 enables the ISA's %4 fast weight load
    (xbus_sel=0xf: 4 columns/cycle instead of 1; see s3_lw.md)."""
    n = 0
    for f in nc.m.functions:
        for bb in f.blocks:
            for inst in bb.instructions:
                tn = type(inst).__name__
                if tn == "InstLdweights":
                    ap = inst.ins[0]
                else:
                    # matmult operand APs must stay 2D (birverifier: "RHS AP
                    # can only have one free dimension")
                    continue
                dims = [list(d) for d in ap.ap]
                if len(dims) == 2 and dims[1][0] == 1 and dims[1][1] % 4 == 0:
                    c = dims[1][1]
                    ap.ap = mybir.VecI64Pair(
                        [dims[0], [4, c // 4], [1, 4]])
                    n += 1
    return n


def _dedup_ldweights(nc):
    """Drop PE InstLdweights whose weights AP is identical to the weights
    currently loaded (i.e. the previous PE ldweights, with only matmuls that
    use those same weights in between). Waits/updates of a dropped ldweights
    fold into the next PE instruction (its matmul) — they are duplicates of
    syncs the kept ldweights already performed, so this only delays them."""
    ndrop = 0
    for f in nc.m.functions:
        for bb in f.blocks:
            new = []
            cur_sig = None
            pe_engine = None
            pend_w, pend_u = [], []
            for inst in bb.instructions:
                tn = type(inst).__name__
                if tn == "InstLdweights":
                    pe_engine = inst.engine
                    sig = _ap_sig(inst.ins[0])
                    if sig == cur_sig:
                        si = inst.sync_info
                        if si is not None:
                            pend_w += list(si.on_wait or [])
                            pend_u += list(si.on_update or [])
                        ndrop += 1
                        continue
                    cur_sig = sig
                elif tn == "InstMatmult":
                    # ins = [moving, stationary]; a matmul on other weights
                    # (shouldn't happen — every mm follows its ldw) resets
                    if len(inst.ins) > 1 and _ap_sig(inst.ins[1]) != cur_sig:
                        cur_sig = None
                if (pend_w or pend_u) and inst.engine == pe_engine:
                    si = inst.sync_info
                    w = list(si.on_wait or []) if si else []
                    u = list(si.on_update or []) if si else []
                    inst.sync_info = bass_rust.SyncInfo(
                        on_wait=pend_w + w, on_update=pend_u + u)
                    pend_w, pend_u = [], []
                new.append(inst)
            assert not pend_w and not pend_u, "trailing dedup sync"
            bb.instructions = new
    return ndrop


def _split_multiwait_insts(nc):
    """This env's walrus CoreV3 codegen accepts only one sync-wait per
    instruction; Tile's tail drain can carry one per DMAHW sem lane.
    Peel extra waits onto same-engine NoOps inserted just before."""
    ctr = 0
    for f in nc.m.functions:
        for bb in f.blocks:
            new = []
            for inst in bb.instructions:
                si = inst.sync_info
                if si is not None and si.on_wait and len(si.on_wait) > 1:
                    waits = list(si.on_wait)
                    for w in waits[:-1]:
                        ctr += 1
                        new.append(bass_rust.InstNoOp(
                            name=f"I-waitsplit-{ctr}",
                            engine=inst.engine,
                            sync_info=bass_rust.SyncInfo(on_wait=[w], on_update=[]),
                        ))
                    inst.sync_info = bass_rust.SyncInfo(
                        on_wait=[waits[-1]], on_update=list(si.on_update or [])
                    )
                new.append(inst)
            bb.instructions = new
    return ctr


def _build(dequant=DEQUANT_ON_DEVICE, split_waits=True):
    nc = bass.Bass()
    # Host-pretiled layouts (see make_in_maps):
    #   xT [P, MT, KO, 128]: xT[p, m, k, i] = x[m*128+i, k*128+p]
    #   wT [P, NOC*KO*OC]:   wT[p, (n, k, j)] = w[n*OC+j, k*128+p]
    #   sT [NOC, KO*OC]:     sT[n, (k, j)]   = scales[n*OC+j, k]
    xd = nc.declare_dram_parameter("xT", [P, MT * KO * P], FP16, isOutput=False)
    wd = nc.declare_dram_parameter("wT", [P, NOC * KO * OC], FP16, isOutput=False)
    if dequant:
        sd = nc.declare_dram_parameter("sT", [NOC * KO, OC], FP16, isOutput=False)
        sel = nc.declare_dram_parameter("sel", [KO, KO * P], FP16, isOutput=False)
    y = nc.declare_dram_parameter("y", [M, OUT], FP16, isOutput=True)

    with tile.TileContext(nc) as tc, ExitStack() as ctx:
        const = ctx.enter_context(tc.tile_pool(name="const", bufs=1))
        xTp = ctx.enter_context(tc.tile_pool(name="xTp", bufs=1))
        wrp = ctx.enter_context(tc.tile_pool(name="wrp", bufs=4))
        scpp = ctx.enter_context(tc.tile_pool(name="scpp", bufs=3))
        psbp = ctx.enter_context(tc.tile_pool(name="psb", bufs=3, space="PSUM"))
        ystg = ctx.enter_context(tc.tile_pool(name="ystg", bufs=4))
        psum = ctx.enter_context(tc.tile_pool(name="psum", bufs=4, space="PSUM"))

        if dequant:
            # one-hot selector stack: selt[:, k, :] is the [32, 128] matrix
            # whose row k is all-ones — lhsT that broadcasts scT row k.
            selt = const.tile([KO, KO, P], FP16)
            nc.gpsimd.dma_start(
                out=selt[:],
                in_=sel[:, :].rearrange("a (k i) -> a k i", k=KO),
            )

        # x^T resident: 8 chunk loads of 1 MiB, 8 KiB/partition each.
        xT = xTp.tile([P, MT, KO, P], FP16)
        CH = KO * P
        for m in range(MT):
            nc.scalar.dma_start(
                out=xT[:, m, :, :],
                in_=xd[:, m * CH:(m + 1) * CH].rearrange("p (k i) -> p k i", k=KO),
            )

        CW = KO * OC

        def emit_load(oc, chunked=False):
            wr = wrp.tile([P, KO, OC], FP16, tag="wr", name=f"wr{oc}")
            if chunked:
                # panel 0 only: 8 ko-sliced DMAs so the first accumulation
                # chain starts after ~2 us instead of the full panel's ~15 us
                # (each matmul ko waits only on its own slice's DMA)
                KC = KO // 8
                for g in range(8):
                    lo = oc * CW + g * KC * OC
                    nc.sync.dma_start(
                        out=wr[:, g * KC:(g + 1) * KC, :],
                        in_=wd[:, lo:lo + KC * OC].rearrange(
                            "p (k j) -> p k j", k=KC),
                    )
            else:
                nc.sync.dma_start(
                    out=wr[:],
                    in_=wd[:, oc * CW:(oc + 1) * CW].rearrange(
                        "p (k j) -> p k j", k=KO),
                )
            if not dequant:
                return (wr, None)
            scp = scpp.tile([KO, OC], FP16, tag="scp", name=f"scp{oc}")
            nc.gpsimd.dma_start(out=scp[:], in_=sd[oc * KO:(oc + 1) * KO, :])
            return (wr, scp)

        def emit_bcast(wrn, scpn, ko):
            psb = psbp.tile([P, OC], FP32, tag="psb", name="psb")
            nc.tensor.matmul(psb[:], selt[:, ko, :], scpn[:],
                             start=True, stop=True)
            nc.vector.tensor_mul(wrn[:, ko, :], wrn[:, ko, :], psb[:])

        def emit_compute(oc, wr, nxt):
            # 32 consecutive matmuls accumulate into ONE psum bank — HW
            # measurements show per-matmul bank alternation costs ~190 ns,
            # so sharing a stationary across two accumulators (which forces
            # alternation) loses more than the saved weight-load.
            osl = slice(oc * OC, (oc + 1) * OC)
            bi = 0
            for m in range(MT):
                pt = psum.tile([P, OC], FP32, name="pt")
                for ko in range(KO):
                    nc.tensor.matmul(
                        pt[:],
                        xT[:, m, ko, :],
                        wr[:, ko, :],
                        start=(ko == 0),
                        stop=(ko == KO - 1),
                    )
                    # next panel's dequant broadcasts, sparse, second half
                    # of the panel only (its wr DMA needs ~15 us of lead)
                    if nxt is not None and m >= MT // 2 and ko % 4 == 3:
                        emit_bcast(nxt[0], nxt[1], bi)
                        bi += 1
                yt = ystg.tile([P, OC], FP16, name="yt")
                nc.scalar.copy(out=yt[:], in_=pt[:])
                nc.scalar.dma_start(out=y[m * P:(m + 1) * P, osl], in_=yt[:])

        lds = [emit_load(0, chunked=True), emit_load(1)]
        if dequant:
            for ko in range(KO):     # panel 0 dequant: standalone prologue
                emit_bcast(lds[0][0], lds[0][1], ko)
        for oc in range(NOC):
            nxt = lds[oc + 1] if (dequant and oc + 1 < NOC) else None
            emit_compute(oc, lds[oc][0], nxt)
            if oc + 2 < NOC:
                lds.append(emit_load(oc + 2))

    if FUSE_LDW:
        _fuse_ldweights(nc)
    if DEDUP_LDW:
        _dedup_ldweights(nc)
    if LDW_FAST_AP:
        _fast_ldweights_aps(nc)
    if THIN_MM_SEMS:
        _thin_mm_sem_updates(nc)
    if split_waits:
        _split_multiwait_insts(nc)
    return nc


def make_in_maps(x, weight, scales, dequant=DEQUANT_ON_DEVICE):
    """Host-side prep: shard + pre-tile into the exact SBUF layouts."""
    xf = np.asarray(x, dtype=np.float16).reshape(NCORES, MT, P, KO, P)
    X = np.ascontiguousarray(xf.transpose(0, 4, 1, 3, 2)).reshape(NCORES, P, -1)
    w = np.asarray(weight, dtype=np.float16)
    s = np.asarray(scales, dtype=np.float16)
    if not dequant:
        # fp16 multiply, same rounding as the reference's jnp fp16 multiply
        w = (w.reshape(OUT, KO, GROUP) * s[:, :, None]).reshape(OUT, IN)
    W = np.ascontiguousarray(
        w.reshape(NOC, OC, KO, P).transpose(3, 0, 2, 1)).reshape(P, -1)
    if dequant:
        # sT[(n, k), j] = scales[n*OC+j, k]
        sT = np.ascontiguousarray(
            s.reshape(NOC, OC, KO).transpose(0, 2, 1)).reshape(NOC * KO, OC)
        # selector stack: sel[i, (k, m)] = 1 if i == k else 0
        sel = np.ascontiguousarray(
            np.broadcast_to(np.eye(KO, dtype=np.float16)[:, :, None], (KO, KO, P))
        ).reshape(KO, KO * P)
    maps = []
    for c in range(NCORES):
        m = {"xT": X[c], "wT": W}
        if dequant:
            m["sT"] = sT
            m["sel"] = sel
        maps.append(m)
    return maps


def _get_runner():
    """Compile once; return a reusable callable mapping per-core input maps
    to per-core output maps (modeled on bass2jax.run_bass_via_pjrt)."""
    global _RUNNER
    if _RUNNER is not None:
        return _RUNNER

    import jax
    from jax.experimental.shard_map import shard_map
    from jax.sharding import Mesh, PartitionSpec
    from concourse import bass2jax

    if LDW_OPT_FLAG:
        _enable_ldw_opt_flag()
    nc = _build()
    bass2jax.install_neuronx_cc_hook()

    partition_name = nc.partition_id_tensor.name if nc.partition_id_tensor else None
    in_names, out_names, out_avals, zero_shapes = [], [], [], []
    for alloc in nc.m.functions[0].allocations:
        if not isinstance(alloc, mybir.MemoryLocationSet):
            continue
        name = alloc.memorylocations[0].name
        if alloc.kind == "ExternalInput":
            if name != partition_name:
                in_names.append(name)
        elif alloc.kind == "ExternalOutput":
            shape = tuple(alloc.tensor_shape)
            dtype = mybir.dt.np(alloc.dtype)
            out_names.append(name)
            out_avals.append(jax.core.ShapedArray(shape, dtype))
            zero_shapes.append((shape, dtype))
    n_params = len(in_names)
    n_outs = len(out_names)
    all_names = in_names + out_names
    if partition_name is not None:
        all_names = all_names + [partition_name]
    donate = tuple(range(n_params, n_params + n_outs))

    def _make_body(reps):
        def _body(*args):
            ins = list(args[:n_params])
            outs = list(args[n_params:n_params + n_outs])
            for _ in range(reps):
                operands = ins + outs
                if partition_name is not None:
                    operands.append(bass2jax.partition_id_tensor())
                outs = list(bass2jax._bass_exec_p.bind(
                    *operands,
                    out_avals=tuple(out_avals),
                    in_names=tuple(all_names),
                    out_names=tuple(out_names),
                    lowering_input_output_aliases=(),
                    sim_require_finite=True,
                    sim_require_nnan=True,
                    nc=nc,
                ))
            return tuple(outs)
        return _body

    devices = jax.devices()[:NCORES]
    mesh = Mesh(np.asarray(devices), ("core",))

    def _make_exec(reps):
        return jax.jit(
            shard_map(
                _make_body(reps),
                mesh=mesh,
                in_specs=(PartitionSpec("core"),) * (n_params + n_outs),
                out_specs=(PartitionSpec("core"),) * n_outs,
                check_rep=False,
            ),
            donate_argnums=donate,
            keep_unused=True,
        )

    sharded = _make_exec(1)
    _exec_cache = {1: sharded}
    from jax.sharding import NamedSharding
    shard = NamedSharding(mesh, PartitionSpec("core"))

    class Runner:
        def __init__(self):
            self.in_names = in_names
            self.out_names = out_names

        def put_inputs(self, in_maps):
            """Concat per-core inputs and place them on the mesh."""
            import jax as _jax
            concat_in = [
                np.concatenate([np.asarray(m[name]) for m in in_maps], axis=0)
                for name in in_names
            ]
            return [_jax.device_put(a, shard) for a in concat_in]

        def fresh_outs(self):
            import jax as _jax
            return [
                _jax.device_put(np.zeros((NCORES * sh[0], *sh[1:]), dt), shard)
                for sh, dt in zero_shapes
            ]

        def exec_dev(self, dev_in, dev_outs, reps=1):
            """Device step(s). dev_outs is donated; returns new out arrays
            (same shape/sharding — reusable as the next call's dev_outs,
            since the kernel overwrites every output element). reps>1
            chains that many NEFF executions inside one dispatch."""
            if reps not in _exec_cache:
                _exec_cache[reps] = _make_exec(reps)
            return _exec_cache[reps](*dev_in, *dev_outs)

        def run(self, in_maps):
            dev_in = self.put_inputs(in_maps)
            out_arrs = self.exec_dev(dev_in, self.fresh_outs())
            return [
                {
                    name: np.asarray(out_arrs[i]).reshape(
                        NCORES, *out_avals[i].shape)[c]
                    for i, name in enumerate(out_names)
                }
                for c in range(NCORES)
            ]

    _RUNNER = Runner()
    return _RUNNER


def kernel(x, weight, scales):
    runner = _get_runner()
    in_maps = make_in_maps(x, weight, scales)
    outs = runner.run(in_maps)
    yf = np.concatenate([outs[c]["y"] for c in range(NCORES)], axis=0)
    return yf.reshape(B, S, OUT).astype(np.float16)


# revision 53
# speedup vs baseline: 1.3334x; 1.3334x over previous
"""Grouped-scale dequant GEMM (AxCoreLinearFP16) on 8 Trainium2 NeuronCores.

y[b,s,o] = sum_i x[b,s,i] * (weight[o,i] * scales[o, i//128])

Strategy: data-parallel over the flattened (b*s) rows — each core gets a
[1024, 4096] x-shard and the full dequantized weight (no collectives).
The scales are folded into the weight on the host (a constant-folding
preprocessing step; np fp16 multiply rounds identically to the
reference's jnp fp16 multiply, so numerics are bit-equal), and all
tensors are pre-tiled on the host into the exact SBUF layout the PE
wants, so every device DMA is a large fully-contiguous transfer.

Device kernel (per core): a wall-to-wall PE matmul stream.
  - x^T resident in SBUF ([128, MT, KO, 128], 64 KiB/partition),
    loaded as 8 x 1 MiB contiguous chunks so m-tile 0 lands early.
  - w^T o-panels ([128, KO, 512], 32 KiB/partition) loaded two panels
    ahead (bufs=3) on their own DMA queue.
  - Per (o-panel, m-tile): 32 matmuls accumulate over the k-chunks
    into ONE PSUM bank ([128, 512] fp32), evicted by a casting ACT
    copy and DMA'd out. 4-bank rotation overlaps eviction.

HW-measured cost law on this toolchain (per N=512 matmul) that shaped
the design — CoreSim's cost model does not see any of these:
  - each matmul is a serial (InstLdweights, InstMatmult) pair; the
    stationary load is NOT hidden under the previous stream
    (~404 ns/MM baseline).
  - rewriting the ldweights AP from [[ps,128],[1,128]] to
    [[ps,128],[4,32],[1,4]] (src num_elems[2]==4) enables the ISA's
    %4 fast weight load, 4 cols/cycle: 404 -> ~341 ns/MM (-128 us).
  - consecutive matmuls that target different PSUM banks pay ~190 ns
    extra — so interleaved accumulation chains, stationary-sharing
    across two accumulators, and interleaved dequant broadcasts all
    LOSE on net. Keep each 32-matmul chain on one bank.
  - the winning form: drop the standalone InstLdweights and set
    ldweights=True on each InstMatmult (true self-loading matmul, ONE
    PE instruction per MM): ~341 -> ~325 ns/MM. Deleting ldweights
    withOUT setting the flag executes with stale weights (wrong
    results), and walrus --enable-ldw-opt=true miscompiles — both
    verified on HW.
  - on-device dequant (PE broadcast + DVE multiply, DEQUANT_ON_DEVICE)
    is kept as a working fallback but costs 150-250 us in bank
    switches + broadcast streams, hence the host fold.

Other toolchain workarounds:
  - walrus here accepts only ONE sync-wait per instruction: extra waits
    are peeled onto same-engine NoOps (_split_multiwait_insts)
  - InstPartitionBroadcast ("ISA wrong length") and broadcast-shaped
    DMAs (step-0 partition APs) are avoided.

Self-contained: hardcodes shapes from the problem spec.
"""

import sys

for _p in ("/opt/trn_rl_repo",):
    if _p not in sys.path:
        sys.path.insert(0, _p)

from contextlib import ExitStack

import numpy as np

import concourse.bass as bass
import concourse.mybir as mybir
import concourse.tile as tile
import bass_rust


FP16 = mybir.dt.float16
FP32 = mybir.dt.float32

P = 128
NCORES = 8
B, S, IN, OUT = 4, 2048, 4096, 4096
GROUP = 128
M = B * S // NCORES          # 1024 rows of x per core
KO = IN // P                 # 32 k-chunks == quant groups
OC = 512                     # o-chunk (matmul free dim)
NOC = OUT // OC              # 8
MT = M // P                  # 8 m-tiles

# True: dequant (scale broadcast + multiply) runs on-device, fused into the
# GEMM pipeline. False: scales are folded into the weight on the host
# (identical fp16 numerics to the reference's jnp fp16 multiply).
DEQUANT_ON_DEVICE = False

# Post-pass: drop an InstLdweights when it reloads the exact weights the PE
# already holds (consecutive matmuls sharing a stationary). HW-verified that
# matmuls use the last-loaded weights; walrus's own ldw-opt path and
# fully-fused self-loading matmuls both miscompute on this toolchain.
DEDUP_LDW = False     # stationary sharing forces PSUM bank alternation: SLOWER
LDW_OPT_FLAG = True   # retry: earlier failures predate mm.ldweights=True
FUSE_LDW = True       # fuse + mm.ldweights=True: single self-loading matmult
LDW_FAST_AP = True    # rewrite ldweights APs to the %4 fast-weight-load form
THIN_MM_SEMS = False  # no measurable win; sem updates ride the commit path

_RUNNER = None


def _enable_ldw_opt_flag():
    """Patch concourse's walrus invocation to enable the LDW optimization
    (the flag is hardcoded off upstream; idempotent)."""
    from concourse import bass_utils
    if getattr(bass_utils.run_command, "_ldw_patched", False):
        return
    orig = bass_utils.run_command

    def run_command_ldw(cmd, **kw):
        cmd = ["--enable-ldw-opt=true" if c == "--enable-ldw-opt=false" else c
               for c in cmd]
        return orig(cmd, **kw)

    run_command_ldw._ldw_patched = True
    bass_utils.run_command = run_command_ldw


def _fuse_ldweights(nc):
    """Drop the standalone InstLdweights that bass splits off each matmul
    (their waits/updates fold into the next same-engine instruction — the
    matmult) and set ldweights=True on every matmult so it performs its own
    weight load (one PE instruction per matmul instead of two)."""
    ndrop = 0
    for f in nc.m.functions:
        for bb in f.blocks:
            new = []
            # pending syncs per engine: the scheduled instruction list
            # interleaves all engines, so a dropped ldweights' waits must
            # land on the next instruction of the SAME engine (its matmult).
            pend = {}
            for inst in bb.instructions:
                tn = type(inst).__name__
                if tn == "InstLdweights":
                    si = inst.sync_info
                    if si is not None and (si.on_wait or si.on_update):
                        w, u = pend.setdefault(inst.engine, ([], []))
                        w += list(si.on_wait or [])
                        u += list(si.on_update or [])
                    ndrop += 1
                    continue
                if tn == "InstMatmult":
                    inst.ldweights = True
                if inst.engine in pend:
                    pw, pu = pend.pop(inst.engine)
                    si = inst.sync_info
                    w = list(si.on_wait or []) if si else []
                    u = list(si.on_update or []) if si else []
                    inst.sync_info = bass_rust.SyncInfo(
                        on_wait=pw + w, on_update=pu + u)
                new.append(inst)
            assert not pend, "trailing ldweights sync"
            bb.instructions = new
    return ndrop


def _thin_mm_sem_updates(nc):
    """Coalesce the per-matmul progress-semaphore increments: drop the +1 on
    non-stop matmuls and make each chain's stop matmul increment by the
    accumulated count. Cumulative semaphore values at every kept update are
    exactly preserved, so all wait thresholds remain correct (a threshold
    that fell mid-chain is now satisfied at the chain's stop — later, never
    earlier). Removes ~97% of PE semaphore-update work."""
    from collections import Counter
    ids = Counter()
    for f in nc.m.functions:
        for bb in f.blocks:
            for inst in bb.instructions:
                if type(inst).__name__ == "InstMatmult":
                    si = inst.sync_info
                    if si is not None:
                        for u in (si.on_update or []):
                            if u.update_mode == "sem-inc" and u.update_value == 1:
                                ids[u.id] += 1
    if not ids:
        return 0
    sem_id = ids.most_common(1)[0][0]
    nthin = 0
    for f in nc.m.functions:
        for bb in f.blocks:
            acc = 0
            last_kept = None
            for inst in bb.instructions:
                if type(inst).__name__ != "InstMatmult":
                    continue
                si = inst.sync_info
                if si is None:
                    continue
                ups = list(si.on_update or [])
                tgt = [u for u in ups
                       if u.id == sem_id and u.update_mode == "sem-inc"
                       and u.update_value == 1]
                if not tgt:
                    continue
                rest = [u for u in ups
                        if not (u.id == sem_id and u.update_mode == "sem-inc"
                                and u.update_value == 1)]
                if inst.stop_tensor_calc:
                    t = tgt[0]
                    # 'sem-inc' bumps by 1 regardless of value; use
                    # 'sem-add-imm' (the mode DMA +=16 updates use) for +N
                    rest.append(bass_rust.SyncUpdate(
                        sync_type=t.sync_type, id=t.id, ant_name=t.ant_name,
                        update_mode="sem-add-imm", update_value=acc + 1,
                        update_reg=None))
                    acc = 0
                    last_kept = inst
                else:
                    acc += 1
                    nthin += 1
                inst.sync_info = bass_rust.SyncInfo(
                    on_wait=list(si.on_wait or []), on_update=rest)
            assert acc == 0, f"unflushed {acc} increments (no trailing stop)"
    return nthin


def _ap_sig(ap):
    return (ap.memref, ap.offset, str(ap.ap), str(ap.dtype))


def _fast_ldweights_aps(nc):
    """Rewrite each PE InstLdweights weights AP from [[pstride, 128], [1, C]]
    to [[pstride, 128], [4, C/4], [1, 4]]. Same element stream, but
    src_mem_pattern.num_elems[2] == 4 enables the ISA's %4 fast weight load
    (xbus_sel=0xf: 4 columns/cycle instead of 1; see s3_lw.md)."""
    n = 0
    for f in nc.m.functions:
        for bb in f.blocks:
            for inst in bb.instructions:
                tn = type(inst).__name__
                if tn == "InstLdweights":
                    ap = inst.ins[0]
                else:
                    # matmult operand APs must stay 2D (birverifier: "RHS AP
                    # can only have one free dimension")
                    continue
                dims = [list(d) for d in ap.ap]
                if len(dims) == 2 and dims[1][0] == 1 and dims[1][1] % 4 == 0:
                    c = dims[1][1]
                    ap.ap = mybir.VecI64Pair(
                        [dims[0], [4, c // 4], [1, 4]])
                    n += 1
    return n


def _dedup_ldweights(nc):
    """Drop PE InstLdweights whose weights AP is identical to the weights
    currently loaded (i.e. the previous PE ldweights, with only matmuls that
    use those same weights in between). Waits/updates of a dropped ldweights
    fold into the next PE instruction (its matmul) — they are duplicates of
    syncs the kept ldweights already performed, so this only delays them."""
    ndrop = 0
    for f in nc.m.functions:
        for bb in f.blocks:
            new = []
            cur_sig = None
            pe_engine = None
            pend_w, pend_u = [], []
            for inst in bb.instructions:
                tn = type(inst).__name__
                if tn == "InstLdweights":
                    pe_engine = inst.engine
                    sig = _ap_sig(inst.ins[0])
                    if sig == cur_sig:
                        si = inst.sync_info
                        if si is not None:
                            pend_w += list(si.on_wait or [])
                            pend_u += list(si.on_update or [])
                        ndrop += 1
                        continue
                    cur_sig = sig
                elif tn == "InstMatmult":
                    # ins = [moving, stationary]; a matmul on other weights
                    # (shouldn't happen — every mm follows its ldw) resets
                    if len(inst.ins) > 1 and _ap_sig(inst.ins[1]) != cur_sig:
                        cur_sig = None
                if (pend_w or pend_u) and inst.engine == pe_engine:
                    si = inst.sync_info
                    w = list(si.on_wait or []) if si else []
                    u = list(si.on_update or []) if si else []
                    inst.sync_info = bass_rust.SyncInfo(
                        on_wait=pend_w + w, on_update=pend_u + u)
                    pend_w, pend_u = [], []
                new.append(inst)
            assert not pend_w and not pend_u, "trailing dedup sync"
            bb.instructions = new
    return ndrop


def _split_multiwait_insts(nc):
    """This env's walrus CoreV3 codegen accepts only one sync-wait per
    instruction; Tile's tail drain can carry one per DMAHW sem lane.
    Peel extra waits onto same-engine NoOps inserted just before."""
    ctr = 0
    for f in nc.m.functions:
        for bb in f.blocks:
            new = []
            for inst in bb.instructions:
                si = inst.sync_info
                if si is not None and si.on_wait and len(si.on_wait) > 1:
                    waits = list(si.on_wait)
                    for w in waits[:-1]:
                        ctr += 1
                        new.append(bass_rust.InstNoOp(
                            name=f"I-waitsplit-{ctr}",
                            engine=inst.engine,
                            sync_info=bass_rust.SyncInfo(on_wait=[w], on_update=[]),
                        ))
                    inst.sync_info = bass_rust.SyncInfo(
                        on_wait=[waits[-1]], on_update=list(si.on_update or [])
                    )
                new.append(inst)
            bb.instructions = new
    return ctr


def _build(dequant=DEQUANT_ON_DEVICE, split_waits=True):
    nc = bass.Bass()
    # Host-pretiled layouts (see make_in_maps):
    #   xT [P, MT, KO, 128]: xT[p, m, k, i] = x[m*128+i, k*128+p]
    #   wT [P, NOC*KO*OC]:   wT[p, (n, k, j)] = w[n*OC+j, k*128+p]
    #   sT [NOC, KO*OC]:     sT[n, (k, j)]   = scales[n*OC+j, k]
    xd = nc.declare_dram_parameter("xT", [P, MT * KO * P], FP16, isOutput=False)
    wd = nc.declare_dram_parameter("wT", [P, NOC * KO * OC], FP16, isOutput=False)
    if dequant:
        sd = nc.declare_dram_parameter("sT", [NOC * KO, OC], FP16, isOutput=False)
        sel = nc.declare_dram_parameter("sel", [KO, KO * P], FP16, isOutput=False)
    y = nc.declare_dram_parameter("y", [M, OUT], FP16, isOutput=True)

    with tile.TileContext(nc) as tc, ExitStack() as ctx:
        const = ctx.enter_context(tc.tile_pool(name="const", bufs=1))
        xTp = ctx.enter_context(tc.tile_pool(name="xTp", bufs=1))
        wrp = ctx.enter_context(tc.tile_pool(name="wrp", bufs=4))
        scpp = ctx.enter_context(tc.tile_pool(name="scpp", bufs=3))
        psbp = ctx.enter_context(tc.tile_pool(name="psb", bufs=3, space="PSUM"))
        ystg = ctx.enter_context(tc.tile_pool(name="ystg", bufs=4))
        psum = ctx.enter_context(tc.tile_pool(name="psum", bufs=4, space="PSUM"))

        if dequant:
            # one-hot selector stack: selt[:, k, :] is the [32, 128] matrix
            # whose row k is all-ones — lhsT that broadcasts scT row k.
            selt = const.tile([KO, KO, P], FP16)
            nc.gpsimd.dma_start(
                out=selt[:],
                in_=sel[:, :].rearrange("a (k i) -> a k i", k=KO),
            )

        # x^T resident: 8 chunk loads of 1 MiB, 8 KiB/partition each.
        xT = xTp.tile([P, MT, KO, P], FP16)
        CH = KO * P
        for m in range(MT):
            nc.scalar.dma_start(
                out=xT[:, m, :, :],
                in_=xd[:, m * CH:(m + 1) * CH].rearrange("p (k i) -> p k i", k=KO),
            )

        CW = KO * OC

        def emit_load(oc, chunked=False):
            wr = wrp.tile([P, KO, OC], FP16, tag="wr", name=f"wr{oc}")
            if chunked:
                # panel 0 only: 8 ko-sliced DMAs so the first accumulation
                # chain starts after ~2 us instead of the full panel's ~15 us
                # (each matmul ko waits only on its own slice's DMA)
                KC = KO // 8
                for g in range(8):
                    lo = oc * CW + g * KC * OC
                    nc.sync.dma_start(
                        out=wr[:, g * KC:(g + 1) * KC, :],
                        in_=wd[:, lo:lo + KC * OC].rearrange(
                            "p (k j) -> p k j", k=KC),
                    )
            else:
                nc.sync.dma_start(
                    out=wr[:],
                    in_=wd[:, oc * CW:(oc + 1) * CW].rearrange(
                        "p (k j) -> p k j", k=KO),
                )
            if not dequant:
                return (wr, None)
            scp = scpp.tile([KO, OC], FP16, tag="scp", name=f"scp{oc}")
            nc.gpsimd.dma_start(out=scp[:], in_=sd[oc * KO:(oc + 1) * KO, :])
            return (wr, scp)

        def emit_bcast(wrn, scpn, ko):
            psb = psbp.tile([P, OC], FP32, tag="psb", name="psb")
            nc.tensor.matmul(psb[:], selt[:, ko, :], scpn[:],
                             start=True, stop=True)
            nc.vector.tensor_mul(wrn[:, ko, :], wrn[:, ko, :], psb[:])

        def emit_compute(oc, wr, nxt):
            # 32 consecutive matmuls accumulate into ONE psum bank — HW
            # measurements show per-matmul bank alternation costs ~190 ns,
            # so sharing a stationary across two accumulators (which forces
            # alternation) loses more than the saved weight-load.
            osl = slice(oc * OC, (oc + 1) * OC)
            bi = 0
            for m in range(MT):
                pt = psum.tile([P, OC], FP32, name="pt")
                for ko in range(KO):
                    nc.tensor.matmul(
                        pt[:],
                        xT[:, m, ko, :],
                        wr[:, ko, :],
                        start=(ko == 0),
                        stop=(ko == KO - 1),
                    )
                    # next panel's dequant broadcasts, sparse, second half
                    # of the panel only (its wr DMA needs ~15 us of lead)
                    if nxt is not None and m >= MT // 2 and ko % 4 == 3:
                        emit_bcast(nxt[0], nxt[1], bi)
                        bi += 1
                yt = ystg.tile([P, OC], FP16, name="yt")
                nc.scalar.copy(out=yt[:], in_=pt[:])
                nc.scalar.dma_start(out=y[m * P:(m + 1) * P, osl], in_=yt[:])

        # NOTE: chunking panel 0's DMA into ko-slices (chunked=True) to start
        # the first chain earlier measured WORSE on HW — per-DMA completion
        # semaphores fire ~us after the data lands, so a fine-grained
        # DMA-to-PE handoff stalls the first chain at every slice boundary.
        lds = [emit_load(0), emit_load(1)]
        if dequant:
            for ko in range(KO):     # panel 0 dequant: standalone prologue
                emit_bcast(lds[0][0], lds[0][1], ko)
        for oc in range(NOC):
            nxt = lds[oc + 1] if (dequant and oc + 1 < NOC) else None
            emit_compute(oc, lds[oc][0], nxt)
            if oc + 2 < NOC:
                lds.append(emit_load(oc + 2))

    if FUSE_LDW:
        _fuse_ldweights(nc)
    if DEDUP_LDW:
        _dedup_ldweights(nc)
    if LDW_FAST_AP:
        _fast_ldweights_aps(nc)
    if THIN_MM_SEMS:
        _thin_mm_sem_updates(nc)
    if split_waits:
        _split_multiwait_insts(nc)
    return nc


def make_in_maps(x, weight, scales, dequant=DEQUANT_ON_DEVICE):
    """Host-side prep: shard + pre-tile into the exact SBUF layouts."""
    xf = np.asarray(x, dtype=np.float16).reshape(NCORES, MT, P, KO, P)
    X = np.ascontiguousarray(xf.transpose(0, 4, 1, 3, 2)).reshape(NCORES, P, -1)
    w = np.asarray(weight, dtype=np.float16)
    s = np.asarray(scales, dtype=np.float16)
    if not dequant:
        # fp16 multiply, same rounding as the reference's jnp fp16 multiply
        w = (w.reshape(OUT, KO, GROUP) * s[:, :, None]).reshape(OUT, IN)
    W = np.ascontiguousarray(
        w.reshape(NOC, OC, KO, P).transpose(3, 0, 2, 1)).reshape(P, -1)
    if dequant:
        # sT[(n, k), j] = scales[n*OC+j, k]
        sT = np.ascontiguousarray(
            s.reshape(NOC, OC, KO).transpose(0, 2, 1)).reshape(NOC * KO, OC)
        # selector stack: sel[i, (k, m)] = 1 if i == k else 0
        sel = np.ascontiguousarray(
            np.broadcast_to(np.eye(KO, dtype=np.float16)[:, :, None], (KO, KO, P))
        ).reshape(KO, KO * P)
    maps = []
    for c in range(NCORES):
        m = {"xT": X[c], "wT": W}
        if dequant:
            m["sT"] = sT
            m["sel"] = sel
        maps.append(m)
    return maps


def _get_runner():
    """Compile once; return a reusable callable mapping per-core input maps
    to per-core output maps (modeled on bass2jax.run_bass_via_pjrt)."""
    global _RUNNER
    if _RUNNER is not None:
        return _RUNNER

    import jax
    from jax.experimental.shard_map import shard_map
    from jax.sharding import Mesh, PartitionSpec
    from concourse import bass2jax

    if LDW_OPT_FLAG:
        _enable_ldw_opt_flag()
    nc = _build()
    bass2jax.install_neuronx_cc_hook()

    partition_name = nc.partition_id_tensor.name if nc.partition_id_tensor else None
    in_names, out_names, out_avals, zero_shapes = [], [], [], []
    for alloc in nc.m.functions[0].allocations:
        if not isinstance(alloc, mybir.MemoryLocationSet):
            continue
        name = alloc.memorylocations[0].name
        if alloc.kind == "ExternalInput":
            if name != partition_name:
                in_names.append(name)
        elif alloc.kind == "ExternalOutput":
            shape = tuple(alloc.tensor_shape)
            dtype = mybir.dt.np(alloc.dtype)
            out_names.append(name)
            out_avals.append(jax.core.ShapedArray(shape, dtype))
            zero_shapes.append((shape, dtype))
    n_params = len(in_names)
    n_outs = len(out_names)
    all_names = in_names + out_names
    if partition_name is not None:
        all_names = all_names + [partition_name]
    donate = tuple(range(n_params, n_params + n_outs))

    def _make_body(reps):
        def _body(*args):
            ins = list(args[:n_params])
            outs = list(args[n_params:n_params + n_outs])
            for _ in range(reps):
                operands = ins + outs
                if partition_name is not None:
                    operands.append(bass2jax.partition_id_tensor())
                outs = list(bass2jax._bass_exec_p.bind(
                    *operands,
                    out_avals=tuple(out_avals),
                    in_names=tuple(all_names),
                    out_names=tuple(out_names),
                    lowering_input_output_aliases=(),
                    sim_require_finite=True,
                    sim_require_nnan=True,
                    nc=nc,
                ))
            return tuple(outs)
        return _body

    devices = jax.devices()[:NCORES]
    mesh = Mesh(np.asarray(devices), ("core",))

    def _make_exec(reps):
        return jax.jit(
            shard_map(
                _make_body(reps),
                mesh=mesh,
                in_specs=(PartitionSpec("core"),) * (n_params + n_outs),
                out_specs=(PartitionSpec("core"),) * n_outs,
                check_rep=False,
            ),
            donate_argnums=donate,
            keep_unused=True,
        )

    sharded = _make_exec(1)
    _exec_cache = {1: sharded}
    from jax.sharding import NamedSharding
    shard = NamedSharding(mesh, PartitionSpec("core"))

    class Runner:
        def __init__(self):
            self.in_names = in_names
            self.out_names = out_names

        def put_inputs(self, in_maps):
            """Concat per-core inputs and place them on the mesh."""
            import jax as _jax
            concat_in = [
                np.concatenate([np.asarray(m[name]) for m in in_maps], axis=0)
                for name in in_names
            ]
            return [_jax.device_put(a, shard) for a in concat_in]

        def fresh_outs(self):
            import jax as _jax
            return [
                _jax.device_put(np.zeros((NCORES * sh[0], *sh[1:]), dt), shard)
                for sh, dt in zero_shapes
            ]

        def exec_dev(self, dev_in, dev_outs, reps=1):
            """Device step(s). dev_outs is donated; returns new out arrays
            (same shape/sharding — reusable as the next call's dev_outs,
            since the kernel overwrites every output element). reps>1
            chains that many NEFF executions inside one dispatch."""
            if reps not in _exec_cache:
                _exec_cache[reps] = _make_exec(reps)
            return _exec_cache[reps](*dev_in, *dev_outs)

        def run(self, in_maps):
            dev_in = self.put_inputs(in_maps)
            out_arrs = self.exec_dev(dev_in, self.fresh_outs())
            return [
                {
                    name: np.asarray(out_arrs[i]).reshape(
                        NCORES, *out_avals[i].shape)[c]
                    for i, name in enumerate(out_names)
                }
                for c in range(NCORES)
            ]

    _RUNNER = Runner()
    return _RUNNER


def kernel(x, weight, scales):
    runner = _get_runner()
    in_maps = make_in_maps(x, weight, scales)
    outs = runner.run(in_maps)
    yf = np.concatenate([outs[c]["y"] for c in range(NCORES)], axis=0)
    return yf.reshape(B, S, OUT).astype(np.float16)
